# revision 29
# baseline (speedup 1.0000x reference)
"""Bass/Tile kernel for nn_BitDanceFP8ScaledLinear (column-parallel over 8 NeuronCores).

y = x @ (weight * weight_scale[:, None]).T + bias
  x: [4, 2048, 4096] f32, weight: [11008, 4096] f32, weight_scale/bias: [11008] f32

Strategy (per core c of 8):
  - weight/scale/bias sharded along out_features (1376 per core); x replicated.
  - Host-side (lossless layout prep only): x is laid out k-major per 256-token
    block as [32 blocks, 128, 32 kchunks, 256 tokens] so every x-block DMA has
    32KB-contiguous per-partition runs; weight shard transposed to wT
    [4096, 1376]; scale/bias replicated to [128, 1376].
  - Device: x blocks are DMA-loaded with an inline fp32->bf16 cast (SWDGE,
    round-to-nearest). The weight streams n-range-major in 96 [128, nsz] f32
    pieces on the Sync HWDGE FIFO, DVE-cast to persistent bf16 tiles.
    Matmuls run bf16 at full PE rate, accumulating fp32 in PSUM
    (psum[tokens=128, outF<=512] += x_tile.T @ w_piece over 32 k-chunks).
  - Startup coverage: the first 4 blocks' groups run k-interleaved 8-wide
    (4 blocks x 2 m-tiles) at each n-range, so the PE consumes each weight
    piece (~1.7us of matmul) faster than it streams (~1.1us): the PE trails
    the stream with no idle instead of stalling on the 63us weight load.
  - Epilogue per PSUM group: y_piece = psum * scale + bias on DVE (per-column
    vectors pre-replicated across partitions), stored via the ScalarE HWDGE
    queue (separate ring from the weight stream - no head-of-line blocking).
  - Host gathers: concatenate core outputs along out_features.
"""

import sys

for _p in ("/opt/trn_rl_repo", "/root/.axon_site/_ro/trn_rl_repo"):
    if _p not in sys.path:
        sys.path.insert(0, _p)

import numpy as np

import concourse.tile as tile
from concourse.tile import add_dep_helper
from concourse import bacc, bass_utils, mybir

B, S, IN, OUT = 4, 2048, 4096, 11008
N_CORES = 8
OUT_SH = OUT // N_CORES  # 1376
TOKENS = B * S  # 8192
P = 128
KO = IN // P  # 32 contraction chunks
T_BLK = 256  # tokens per x block
NBLK = TOKENS // T_BLK  # 32
NB = T_BLK // P  # m-tiles per block (2)
N_SPLITS = [(0, 512), (512, 512), (1024, 352)]  # OUT_SH split into PSUM-bank-sized pieces
EARLY = 4  # blocks covered by the startup interleave

_cache = {}


def _build_program():
    nc = bacc.Bacc("TRN2", target_bir_lowering=False, debug=False, num_devices=N_CORES)

    xq = nc.dram_tensor("xq", [NBLK, P, KO, T_BLK], mybir.dt.float32, kind="ExternalInput").ap()
    wT = nc.dram_tensor("wT", [IN, OUT_SH], mybir.dt.float32, kind="ExternalInput").ap()
    sc = nc.dram_tensor("scale_rep", [P, OUT_SH], mybir.dt.float32, kind="ExternalInput").ap()
    bi = nc.dram_tensor("bias_rep", [P, OUT_SH], mybir.dt.float32, kind="ExternalInput").ap()
    y = nc.dram_tensor("y", [TOKENS, OUT_SH], mybir.dt.float32, kind="ExternalOutput").ap()

    wT_t = wT.rearrange("(ko ki) n -> ki ko n", ki=P)  # [128, 32, 1376]

    with tile.TileContext(nc) as tc:
        with (
            tc.tile_pool(name="const", bufs=1) as const,
            tc.tile_pool(name="wstage", bufs=6) as wstage,
            tc.tile_pool(name="xp", bufs=2) as xp,
            tc.tile_pool(name="outp", bufs=6) as outp,
            tc.tile_pool(name="psum", bufs=8, space="PSUM") as psp,
        ):
            # Blocks 0-1 arrive as interleaved quarter-tiles on the otherwise
            # empty SWDGE queue (fp32->bf16 cast inline, 8KB-contiguous runs):
            # fine-grained deps let the PE start at the first w piece, and the
            # total x-early bytes (8MB) spread over the w stream keep the
            # piece-arrival rate at ~the PE consumption rate.
            QK = KO // 4  # k-chunks per quarter
            xquart = {}  # (blk, q) -> tile
            for q in range(4):
                for blk in range(2):
                    xt = xp.tile([P, QK, T_BLK], mybir.dt.bfloat16, name=f"xq_{blk}_{q}", bufs=1)
                    nc.gpsimd.dma_start(xt[:], xq[blk, :, q * QK : (q + 1) * QK, :])
                    xquart[(blk, q)] = xt

            def xslice(blk, k, mi):
                if blk < 2:
                    return xquart[(blk, k // QK)][:, k % QK, mi * P : (mi + 1) * P]
                return xbs[blk][:, k, mi * P : (mi + 1) * P]

            xbs = {}

            # Weight: n-range-major stream of 96 pieces on the Sync HWDGE
            # FIFO, staged f32 then DVE-cast into persistent bf16 tiles.
            wbk = {}
            wcast = {}

            def emit_w_range(nr):
                n0, nsz = N_SPLITS[nr]
                for k in range(KO):
                    wst = wstage.tile([P, 512], mybir.dt.float32, name="wst")
                    nc.sync.dma_start(wst[:, :nsz], wT_t[:, k, n0 : n0 + nsz])
                    wbt = const.tile([P, nsz], mybir.dt.bfloat16, name=f"wb_{nr}_{k}")
                    wcast[(nr, k)] = nc.vector.tensor_copy(wbt[:], wst[:, :nsz])
                    wbk[(nr, k)] = wbt

            emit_w_range(0)
            # scale/bias ride the ScalarE HWDGE ring, off the critical w FIFO
            sct = const.tile([P, OUT_SH], mybir.dt.float32)
            nc.scalar.dma_start(sct[:], sc[:])
            bit = const.tile([P, OUT_SH], mybir.dt.float32)
            nc.scalar.dma_start(bit[:], bi[:])
            emit_w_range(1)
            emit_w_range(2)

            def evict_store(ps, blk, mi, nr):
                """y_piece = psum * scale + bias; store via ScalarE HWDGE."""
                n0, nsz = N_SPLITS[nr]
                op = outp.tile([P, 512], mybir.dt.float32, name="op")[:, :nsz]
                nc.vector.tensor_mul(op, ps, sct[:, n0 : n0 + nsz])
                nc.vector.tensor_add(op, op, bit[:, n0 : n0 + nsz])
                trow = blk * T_BLK + mi * P
                nc.scalar.dma_start(y[trow : trow + P, n0 : n0 + nsz], op)

            # x blocks 2-3: full-tile SWDGE loads gated into nr1's surplus
            # window (the 4-wide nr1 interleave over-covers the stream 2.7x,
            # so their bandwidth theft is absorbed there).
            for blk in (2, 3):
                xb = xp.tile([P, KO, T_BLK], mybir.dt.bfloat16, name="xb")
                xdma = nc.gpsimd.dma_start(xb[:], xq[blk])
                gate = {2: (1, 4), 3: (1, 20)}[blk]
                add_dep_helper(xdma.ins, wcast[gate].ins, sync=True,
                               reason="pace x prefetch behind w stream")
                xbs[blk] = xb

            def xsl(blk, k, mi):
                if blk < 2:
                    return xquart[(blk, k // QK)][:, k % QK, mi * P : (mi + 1) * P]
                return xbs[blk][:, k, mi * P : (mi + 1) * P]

            # ---- startup phase. nr0/nr1: blocks 0-1 k-interleaved 4-wide,
            # trailing the weight stream.
            def interleaved(nr, blocks):
                nsz = N_SPLITS[nr][1]
                groups = [(blk, mi) for blk in blocks for mi in range(NB)]
                pss = [psp.tile([P, 512], mybir.dt.float32, name="ps")[:, :nsz] for _ in groups]
                for k in range(KO):
                    for g, (blk, mi) in enumerate(groups):
                        nc.tensor.matmul(
                            pss[g],
                            xsl(blk, k, mi),
                            wbk[(nr, k)][:],
                            start=(k == 0),
                            stop=(k == KO - 1),
                        )
                for g, (blk, mi) in enumerate(groups):
                    evict_store(pss[g], blk, mi, nr)

            interleaved(0, (0, 1))
            interleaved(1, (0, 1))

            # blocks 2-3: nr0/nr1 dense while the nr2 pieces stream in — by
            # the time the nr2 interleave starts everything is resident.
            for blk in (2, 3):
                for mi in range(NB):
                    for nr in (0, 1):
                        nsz = N_SPLITS[nr][1]
                        ps = psp.tile([P, 512], mybir.dt.float32, name="ps")[:, :nsz]
                        for k in range(KO):
                            nc.tensor.matmul(
                                ps,
                                xbs[blk][:, k, mi * P : (mi + 1) * P],
                                wbk[(nr, k)][:],
                                start=(k == 0),
                                stop=(k == KO - 1),
                            )
                        evict_store(ps, blk, mi, nr)

            # nr2: blocks 2-3 trickle the remaining stream first (so x2/x3's
            # last readers retire early and free the x slots for block 4),
            # then blocks 0-1 run nr2 dense.
            interleaved(2, (2, 3))
            for blk in (0, 1):
                for mi in range(NB):
                    nsz = N_SPLITS[2][1]
                    ps = psp.tile([P, 512], mybir.dt.float32, name="ps")[:, :nsz]
                    for k in range(KO):
                        nc.tensor.matmul(
                            ps,
                            xsl(blk, k, mi),
                            wbk[(2, k)][:],
                            start=(k == 0),
                            stop=(k == KO - 1),
                        )
                    evict_store(ps, blk, mi, 2)

            # ---- steady state: blocks 4..NBLK-1
            for blk in range(4, NBLK):
                xb = xp.tile([P, KO, T_BLK], mybir.dt.bfloat16, name="xb")
                xdma = nc.gpsimd.dma_start(xb[:], xq[blk])
                gate = {4: (2, 31), 5: (2, 31)}.get(blk)
                if gate is not None:
                    add_dep_helper(xdma.ins, wcast[gate].ins, sync=True,
                                   reason="pace x prefetch behind w stream")
                for mi in range(NB):
                    for nr in range(len(N_SPLITS)):
                        nsz = N_SPLITS[nr][1]
                        ps = psp.tile([P, 512], mybir.dt.float32, name="ps")[:, :nsz]
                        for k in range(KO):
                            nc.tensor.matmul(
                                ps,
                                xb[:, k, mi * P : (mi + 1) * P],
                                wbk[(nr, k)][:],
                                start=(k == 0),
                                stop=(k == KO - 1),
                            )
                        evict_store(ps, blk, mi, nr)

    nc.compile()
    return nc


def _prep_inputs(x, weight, weight_scale, bias):
    x2 = np.ascontiguousarray(x, dtype=np.float32).reshape(TOKENS, IN)
    # [blk, ki, ko, t]: xq[b, ki, ko, t] = x[b*T_BLK + t, ko*P + ki]
    xq = np.ascontiguousarray(
        x2.reshape(NBLK, T_BLK, KO, P).transpose(0, 3, 2, 1)
    )
    in_maps = []
    for c in range(N_CORES):
        lo, hi = c * OUT_SH, (c + 1) * OUT_SH
        wTc = np.ascontiguousarray(weight[lo:hi, :].astype(np.float32, copy=False).T)
        scc = np.ascontiguousarray(
            np.broadcast_to(weight_scale[lo:hi].astype(np.float32, copy=False)[None, :], (P, OUT_SH))
        )
        bic = np.ascontiguousarray(
            np.broadcast_to(bias[lo:hi].astype(np.float32, copy=False)[None, :], (P, OUT_SH))
        )
        in_maps.append({"xq": xq, "wT": wTc, "scale_rep": scc, "bias_rep": bic})
    return in_maps


def kernel(x, weight, weight_scale, bias, _trace=False):
    if "nc" not in _cache:
        _cache["nc"] = _build_program()
    nc = _cache["nc"]
    in_maps = _prep_inputs(x, weight, weight_scale, bias)
    res = bass_utils.run_bass_kernel_spmd(
        nc, in_maps, core_ids=list(range(N_CORES)), trace=_trace
    )
    _cache["last_result"] = res
    out = np.concatenate([res.results[c]["y"] for c in range(N_CORES)], axis=1)
    return out.reshape(B, S, OUT)


# revision 34
# speedup vs baseline: 1.0024x; 1.0024x over previous
"""Bass/Tile kernel for nn_BitDanceFP8ScaledLinear (column-parallel over 8 NeuronCores).

y = x @ (weight * weight_scale[:, None]).T + bias
  x: [4, 2048, 4096] f32, weight: [11008, 4096] f32, weight_scale/bias: [11008] f32

Strategy (per core c of 8):
  - weight/scale/bias sharded along out_features (1376 per core); x replicated.
  - Host-side (lossless layout prep only): x is laid out k-major per 256-token
    block as [32 blocks, 128, 32 kchunks, 256 tokens] so every x-block DMA has
    32KB-contiguous per-partition runs; weight shard transposed to wT
    [4096, 1376]; scale/bias replicated to [128, 1376].
  - Device: x blocks are DMA-loaded with an inline fp32->bf16 cast (SWDGE,
    round-to-nearest). The weight streams n-range-major in 96 [128, nsz] f32
    pieces on the Sync HWDGE FIFO, DVE-cast to persistent bf16 tiles.
    Matmuls run bf16 at full PE rate, accumulating fp32 in PSUM
    (psum[tokens=128, outF<=512] += x_tile.T @ w_piece over 32 k-chunks).
  - Startup coverage: the first 4 blocks' groups run k-interleaved 8-wide
    (4 blocks x 2 m-tiles) at each n-range, so the PE consumes each weight
    piece (~1.7us of matmul) faster than it streams (~1.1us): the PE trails
    the stream with no idle instead of stalling on the 63us weight load.
  - Epilogue per PSUM group: y_piece = psum * scale + bias on DVE (per-column
    vectors pre-replicated across partitions), stored via the ScalarE HWDGE
    queue (separate ring from the weight stream - no head-of-line blocking).
  - Host gathers: concatenate core outputs along out_features.
"""

import sys

for _p in ("/opt/trn_rl_repo", "/root/.axon_site/_ro/trn_rl_repo"):
    if _p not in sys.path:
        sys.path.insert(0, _p)

import numpy as np

import concourse.tile as tile
from concourse.tile import add_dep_helper
from concourse import bacc, bass_utils, mybir

B, S, IN, OUT = 4, 2048, 4096, 11008
N_CORES = 8
OUT_SH = OUT // N_CORES  # 1376
TOKENS = B * S  # 8192
P = 128
KO = IN // P  # 32 contraction chunks
T_BLK = 256  # tokens per x block
NBLK = TOKENS // T_BLK  # 32
NB = T_BLK // P  # m-tiles per block (2)
N_SPLITS = [(0, 512), (512, 512), (1024, 352)]  # OUT_SH split into PSUM-bank-sized pieces
EARLY = 4  # blocks covered by the startup interleave

_cache = {}


def _build_program():
    nc = bacc.Bacc("TRN2", target_bir_lowering=False, debug=False, num_devices=N_CORES)

    xq = nc.dram_tensor("xq", [NBLK, P, KO, T_BLK], mybir.dt.float32, kind="ExternalInput").ap()
    wT = nc.dram_tensor("wT", [IN, OUT_SH], mybir.dt.float32, kind="ExternalInput").ap()
    sc = nc.dram_tensor("scale_rep", [P, OUT_SH], mybir.dt.float32, kind="ExternalInput").ap()
    bi = nc.dram_tensor("bias_rep", [P, OUT_SH], mybir.dt.float32, kind="ExternalInput").ap()
    y = nc.dram_tensor("y", [TOKENS, OUT_SH], mybir.dt.float32, kind="ExternalOutput").ap()

    wT_t = wT.rearrange("(ko ki) n -> ki ko n", ki=P)  # [128, 32, 1376]

    with tile.TileContext(nc) as tc:
        with (
            tc.tile_pool(name="const", bufs=1) as const,
            tc.tile_pool(name="wstage", bufs=5) as wstage,
            tc.tile_pool(name="xp", bufs=2) as xp,
            tc.tile_pool(name="outp", bufs=5) as outp,
            tc.tile_pool(name="psum", bufs=8, space="PSUM") as psp,
        ):
            # Blocks 0-1 arrive as interleaved quarter-tiles on the otherwise
            # empty SWDGE queue (fp32->bf16 cast inline, 8KB-contiguous runs):
            # fine-grained deps let the PE start at the first w piece, and the
            # total x-early bytes (8MB) spread over the w stream keep the
            # piece-arrival rate at ~the PE consumption rate.
            QK = KO // 4  # k-chunks per quarter
            xquart = {}  # (blk, q) -> tile
            for q in range(4):
                for blk in range(2):
                    xt = xp.tile([P, QK, T_BLK], mybir.dt.bfloat16, name=f"xq_{blk}_{q}", bufs=1)
                    nc.gpsimd.dma_start(xt[:], xq[blk, :, q * QK : (q + 1) * QK, :])
                    xquart[(blk, q)] = xt

            def xslice(blk, k, mi):
                if blk < 2:
                    return xquart[(blk, k // QK)][:, k % QK, mi * P : (mi + 1) * P]
                return xbs[blk][:, k, mi * P : (mi + 1) * P]

            xbs = {}

            # Weight: n-range-major stream of 96 pieces on the Sync HWDGE
            # FIFO, staged f32 then DVE-cast into persistent bf16 tiles.
            wbk = {}
            wcast = {}

            def emit_w_range(nr):
                n0, nsz = N_SPLITS[nr]
                for k in range(KO):
                    wst = wstage.tile([P, 512], mybir.dt.float32, name="wst")
                    nc.sync.dma_start(wst[:, :nsz], wT_t[:, k, n0 : n0 + nsz])
                    wbt = const.tile([P, nsz], mybir.dt.bfloat16, name=f"wb_{nr}_{k}")
                    wcast[(nr, k)] = nc.vector.tensor_copy(wbt[:], wst[:, :nsz])
                    wbk[(nr, k)] = wbt

            emit_w_range(0)
            # scale/bias ride the ScalarE HWDGE ring, off the critical w FIFO
            sct = const.tile([P, OUT_SH], mybir.dt.float32)
            nc.scalar.dma_start(sct[:], sc[:])
            bit = const.tile([P, OUT_SH], mybir.dt.float32)
            nc.scalar.dma_start(bit[:], bi[:])
            emit_w_range(1)
            # x3 rides the Sync FIFO between nr1 and nr2 at full rate: its
            # transfer window is covered by block-2's dense work (x2-only),
            # and the nr2 pieces stream clean of SWDGE interference after it.
            xb3 = xp.tile([P, KO, T_BLK], mybir.dt.bfloat16, name="xb")
            for h in range(4):
                xst = wstage.tile([P, 8, T_BLK], mybir.dt.float32, name="x3stage", bufs=1)
                nc.sync.dma_start(xst[:], xq[3, :, h * 8 : (h + 1) * 8, :])
                nc.vector.tensor_copy(xb3[:, h * 8 : (h + 1) * 8, :], xst[:])
            xbs[3] = xb3
            emit_w_range(2)

            def evict_store(ps, blk, mi, nr):
                """y_piece = psum * scale + bias; store via ScalarE HWDGE."""
                n0, nsz = N_SPLITS[nr]
                op = outp.tile([P, 512], mybir.dt.float32, name="op")[:, :nsz]
                nc.vector.tensor_mul(op, ps, sct[:, n0 : n0 + nsz])
                nc.vector.tensor_add(op, op, bit[:, n0 : n0 + nsz])
                trow = blk * T_BLK + mi * P
                nc.scalar.dma_start(y[trow : trow + P, n0 : n0 + nsz], op)

            # x blocks 2-3: full-tile SWDGE loads gated into nr1's surplus
            # window (the 4-wide nr1 interleave over-covers the stream 2.7x,
            # so their bandwidth theft is absorbed there).
            for blk in (2,):
                xb = xp.tile([P, KO, T_BLK], mybir.dt.bfloat16, name="xb")
                xdma = nc.gpsimd.dma_start(xb[:], xq[blk])
                add_dep_helper(xdma.ins, wcast[(1, 4)].ins, sync=True,
                               reason="pace x prefetch behind w stream")
                xbs[blk] = xb

            def xsl(blk, k, mi):
                if blk < 2:
                    return xquart[(blk, k // QK)][:, k % QK, mi * P : (mi + 1) * P]
                return xbs[blk][:, k, mi * P : (mi + 1) * P]

            # ---- startup phase. nr0/nr1: blocks 0-1 k-interleaved 4-wide,
            # trailing the weight stream.
            def interleaved(nr, blocks):
                nsz = N_SPLITS[nr][1]
                groups = [(blk, mi) for blk in blocks for mi in range(NB)]
                pss = [psp.tile([P, 512], mybir.dt.float32, name="ps")[:, :nsz] for _ in groups]
                for k in range(KO):
                    for g, (blk, mi) in enumerate(groups):
                        nc.tensor.matmul(
                            pss[g],
                            xsl(blk, k, mi),
                            wbk[(nr, k)][:],
                            start=(k == 0),
                            stop=(k == KO - 1),
                        )
                for g, (blk, mi) in enumerate(groups):
                    evict_store(pss[g], blk, mi, nr)

            interleaved(0, (0, 1))
            interleaved(1, (0, 1))

            # blocks 2-3: nr0/nr1 dense while the nr2 pieces stream in — by
            # the time the nr2 interleave starts everything is resident.
            for blk in (2, 3):
                for mi in range(NB):
                    for nr in (0, 1):
                        nsz = N_SPLITS[nr][1]
                        ps = psp.tile([P, 512], mybir.dt.float32, name="ps")[:, :nsz]
                        for k in range(KO):
                            nc.tensor.matmul(
                                ps,
                                xbs[blk][:, k, mi * P : (mi + 1) * P],
                                wbk[(nr, k)][:],
                                start=(k == 0),
                                stop=(k == KO - 1),
                            )
                        evict_store(ps, blk, mi, nr)

            # nr2: blocks 2-3 trickle the remaining stream first (so x2/x3's
            # last readers retire early and free the x slots for block 4),
            # then blocks 0-1 run nr2 dense.
            interleaved(2, (2, 3))
            for blk in (0, 1):
                for mi in range(NB):
                    nsz = N_SPLITS[2][1]
                    ps = psp.tile([P, 512], mybir.dt.float32, name="ps")[:, :nsz]
                    for k in range(KO):
                        nc.tensor.matmul(
                            ps,
                            xsl(blk, k, mi),
                            wbk[(2, k)][:],
                            start=(k == 0),
                            stop=(k == KO - 1),
                        )
                    evict_store(ps, blk, mi, 2)

            # ---- steady state: blocks 4..NBLK-1
            for blk in range(4, NBLK):
                xb = xp.tile([P, KO, T_BLK], mybir.dt.bfloat16, name="xb")
                xdma = nc.gpsimd.dma_start(xb[:], xq[blk])
                gate = {4: (2, 8), 5: (2, 31)}.get(blk)
                if gate is not None:
                    add_dep_helper(xdma.ins, wcast[gate].ins, sync=True,
                                   reason="pace x prefetch behind w stream")
                for mi in range(NB):
                    for nr in range(len(N_SPLITS)):
                        nsz = N_SPLITS[nr][1]
                        ps = psp.tile([P, 512], mybir.dt.float32, name="ps")[:, :nsz]
                        for k in range(KO):
                            nc.tensor.matmul(
                                ps,
                                xb[:, k, mi * P : (mi + 1) * P],
                                wbk[(nr, k)][:],
                                start=(k == 0),
                                stop=(k == KO - 1),
                            )
                        evict_store(ps, blk, mi, nr)

    nc.compile()
    return nc


def _prep_inputs(x, weight, weight_scale, bias):
    x2 = np.ascontiguousarray(x, dtype=np.float32).reshape(TOKENS, IN)
    # [blk, ki, ko, t]: xq[b, ki, ko, t] = x[b*T_BLK + t, ko*P + ki]
    xq = np.ascontiguousarray(
        x2.reshape(NBLK, T_BLK, KO, P).transpose(0, 3, 2, 1)
    )
    in_maps = []
    for c in range(N_CORES):
        lo, hi = c * OUT_SH, (c + 1) * OUT_SH
        wTc = np.ascontiguousarray(weight[lo:hi, :].astype(np.float32, copy=False).T)
        scc = np.ascontiguousarray(
            np.broadcast_to(weight_scale[lo:hi].astype(np.float32, copy=False)[None, :], (P, OUT_SH))
        )
        bic = np.ascontiguousarray(
            np.broadcast_to(bias[lo:hi].astype(np.float32, copy=False)[None, :], (P, OUT_SH))
        )
        in_maps.append({"xq": xq, "wT": wTc, "scale_rep": scc, "bias_rep": bic})
    return in_maps


def kernel(x, weight, weight_scale, bias, _trace=False):
    if "nc" not in _cache:
        _cache["nc"] = _build_program()
    nc = _cache["nc"]
    in_maps = _prep_inputs(x, weight, weight_scale, bias)
    res = bass_utils.run_bass_kernel_spmd(
        nc, in_maps, core_ids=list(range(N_CORES)), trace=_trace
    )
    _cache["last_result"] = res
    out = np.concatenate([res.results[c]["y"] for c in range(N_CORES)], axis=1)
    return out.reshape(B, S, OUT)


# revision 37
# speedup vs baseline: 1.0048x; 1.0024x over previous
"""Bass/Tile kernel for nn_BitDanceFP8ScaledLinear (column-parallel over 8 NeuronCores).

y = x @ (weight * weight_scale[:, None]).T + bias
  x: [4, 2048, 4096] f32, weight: [11008, 4096] f32, weight_scale/bias: [11008] f32

Strategy (per core c of 8):
  - weight/scale/bias sharded along out_features (1376 per core); x replicated.
  - Host-side (lossless layout prep only): x is laid out k-major per 256-token
    block as [32 blocks, 128, 32 kchunks, 256 tokens] so every x-block DMA has
    32KB-contiguous per-partition runs; weight shard transposed to wT
    [4096, 1376]; scale/bias replicated to [128, 1376].
  - Device: x blocks are DMA-loaded with an inline fp32->bf16 cast (SWDGE,
    round-to-nearest). The weight streams n-range-major in 96 [128, nsz] f32
    pieces on the Sync HWDGE FIFO, DVE-cast to persistent bf16 tiles.
    Matmuls run bf16 at full PE rate, accumulating fp32 in PSUM
    (psum[tokens=128, outF<=512] += x_tile.T @ w_piece over 32 k-chunks).
  - Startup coverage: the first 4 blocks' groups run k-interleaved 8-wide
    (4 blocks x 2 m-tiles) at each n-range, so the PE consumes each weight
    piece (~1.7us of matmul) faster than it streams (~1.1us): the PE trails
    the stream with no idle instead of stalling on the 63us weight load.
  - Epilogue per PSUM group: y_piece = psum * scale + bias on DVE (per-column
    vectors pre-replicated across partitions), stored via the ScalarE HWDGE
    queue (separate ring from the weight stream - no head-of-line blocking).
  - Host gathers: concatenate core outputs along out_features.
"""

import sys

for _p in ("/opt/trn_rl_repo", "/root/.axon_site/_ro/trn_rl_repo"):
    if _p not in sys.path:
        sys.path.insert(0, _p)

import numpy as np

import concourse.tile as tile
from concourse.tile import add_dep_helper
from concourse import bacc, bass_utils, mybir

B, S, IN, OUT = 4, 2048, 4096, 11008
N_CORES = 8
OUT_SH = OUT // N_CORES  # 1376
TOKENS = B * S  # 8192
P = 128
KO = IN // P  # 32 contraction chunks
T_BLK = 256  # tokens per x block
NBLK = TOKENS // T_BLK  # 32
NB = T_BLK // P  # m-tiles per block (2)
N_SPLITS = [(0, 512), (512, 512), (1024, 352)]  # OUT_SH split into PSUM-bank-sized pieces
EARLY = 4  # blocks covered by the startup interleave

_cache = {}


def _build_program():
    nc = bacc.Bacc("TRN2", target_bir_lowering=False, debug=False, num_devices=N_CORES)

    xq = nc.dram_tensor("xq", [NBLK, P, KO, T_BLK], mybir.dt.float32, kind="ExternalInput").ap()
    wT = nc.dram_tensor("wT", [IN, OUT_SH], mybir.dt.float32, kind="ExternalInput").ap()
    sc = nc.dram_tensor("scale_rep", [P, OUT_SH], mybir.dt.float32, kind="ExternalInput").ap()
    bi = nc.dram_tensor("bias_rep", [P, OUT_SH], mybir.dt.float32, kind="ExternalInput").ap()
    y = nc.dram_tensor("y", [TOKENS, OUT_SH], mybir.dt.float32, kind="ExternalOutput").ap()

    wT_t = wT.rearrange("(ko ki) n -> ki ko n", ki=P)  # [128, 32, 1376]

    with tile.TileContext(nc) as tc:
        with (
            tc.tile_pool(name="const", bufs=1) as const,
            tc.tile_pool(name="wstage", bufs=5) as wstage,
            tc.tile_pool(name="xp", bufs=2) as xp,
            tc.tile_pool(name="outp", bufs=5) as outp,
            tc.tile_pool(name="psum", bufs=8, space="PSUM") as psp,
        ):
            # Blocks 0-1 arrive as interleaved quarter-tiles on the otherwise
            # empty SWDGE queue (fp32->bf16 cast inline, 8KB-contiguous runs):
            # fine-grained deps let the PE start at the first w piece, and the
            # total x-early bytes (8MB) spread over the w stream keep the
            # piece-arrival rate at ~the PE consumption rate.
            QK = KO // 4  # k-chunks per quarter
            xquart = {}  # (blk, q) -> tile
            for q in range(4):
                for blk in range(2):
                    xt = xp.tile([P, QK, T_BLK], mybir.dt.bfloat16, name=f"xq_{blk}_{q}", bufs=1)
                    nc.gpsimd.dma_start(xt[:], xq[blk, :, q * QK : (q + 1) * QK, :])
                    xquart[(blk, q)] = xt

            def xslice(blk, k, mi):
                if blk < 2:
                    return xquart[(blk, k // QK)][:, k % QK, mi * P : (mi + 1) * P]
                return xbs[blk][:, k, mi * P : (mi + 1) * P]

            xbs = {}

            # Weight: n-range-major stream of 96 pieces on the Sync HWDGE
            # FIFO, staged f32 then DVE-cast into persistent bf16 tiles.
            wbk = {}
            wcast = {}

            def emit_w_range(nr):
                n0, nsz = N_SPLITS[nr]
                for k in range(KO):
                    wst = wstage.tile([P, 512], mybir.dt.float32, name="wst")
                    nc.sync.dma_start(wst[:, :nsz], wT_t[:, k, n0 : n0 + nsz])
                    wbt = const.tile([P, nsz], mybir.dt.bfloat16, name=f"wb_{nr}_{k}")
                    wcast[(nr, k)] = nc.vector.tensor_copy(wbt[:], wst[:, :nsz])
                    wbk[(nr, k)] = wbt

            emit_w_range(0)
            # scale/bias ride the ScalarE HWDGE ring, off the critical w FIFO
            sct = const.tile([P, OUT_SH], mybir.dt.float32)
            nc.scalar.dma_start(sct[:], sc[:])
            bit = const.tile([P, OUT_SH], mybir.dt.float32)
            nc.scalar.dma_start(bit[:], bi[:])
            emit_w_range(1)
            # x3 rides the Sync FIFO between nr1 and nr2 at full rate: its
            # transfer window is covered by block-2's dense work (x2-only),
            # and the nr2 pieces stream clean of SWDGE interference after it.
            xb3 = xp.tile([P, KO, T_BLK], mybir.dt.bfloat16, name="xb")
            for h in range(4):
                xst = wstage.tile([P, 8, T_BLK], mybir.dt.float32, name="x3stage", bufs=1)
                nc.sync.dma_start(xst[:], xq[3, :, h * 8 : (h + 1) * 8, :])
                nc.vector.tensor_copy(xb3[:, h * 8 : (h + 1) * 8, :], xst[:])
            xbs[3] = xb3
            emit_w_range(2)

            def evict_store(ps, blk, mi, nr):
                """y_piece = psum * scale + bias; store via ScalarE HWDGE."""
                n0, nsz = N_SPLITS[nr]
                op = outp.tile([P, 512], mybir.dt.float32, name="op")[:, :nsz]
                nc.vector.tensor_mul(op, ps, sct[:, n0 : n0 + nsz])
                nc.vector.tensor_add(op, op, bit[:, n0 : n0 + nsz])
                trow = blk * T_BLK + mi * P
                nc.scalar.dma_start(y[trow : trow + P, n0 : n0 + nsz], op)

            # x blocks 2-3: full-tile SWDGE loads gated into nr1's surplus
            # window (the 4-wide nr1 interleave over-covers the stream 2.7x,
            # so their bandwidth theft is absorbed there).
            for blk in (2,):
                xb = xp.tile([P, KO, T_BLK], mybir.dt.bfloat16, name="xb")
                xdma = nc.gpsimd.dma_start(xb[:], xq[blk])
                add_dep_helper(xdma.ins, wcast[(1, 1)].ins, sync=True,
                               reason="pace x prefetch behind w stream")
                xbs[blk] = xb

            def xsl(blk, k, mi):
                if blk < 2:
                    return xquart[(blk, k // QK)][:, k % QK, mi * P : (mi + 1) * P]
                return xbs[blk][:, k, mi * P : (mi + 1) * P]

            # ---- startup phase. nr0/nr1: blocks 0-1 k-interleaved 4-wide,
            # trailing the weight stream.
            def interleaved(nr, blocks):
                nsz = N_SPLITS[nr][1]
                groups = [(blk, mi) for blk in blocks for mi in range(NB)]
                pss = [psp.tile([P, 512], mybir.dt.float32, name="ps")[:, :nsz] for _ in groups]
                for k in range(KO):
                    for g, (blk, mi) in enumerate(groups):
                        nc.tensor.matmul(
                            pss[g],
                            xsl(blk, k, mi),
                            wbk[(nr, k)][:],
                            start=(k == 0),
                            stop=(k == KO - 1),
                        )
                for g, (blk, mi) in enumerate(groups):
                    evict_store(pss[g], blk, mi, nr)

            interleaved(0, (0, 1))
            interleaved(1, (0, 1))

            # blocks 2-3: nr0/nr1 dense while the nr2 pieces stream in — by
            # the time the nr2 interleave starts everything is resident.
            for blk in (2, 3):
                for mi in range(NB):
                    for nr in (0, 1):
                        nsz = N_SPLITS[nr][1]
                        ps = psp.tile([P, 512], mybir.dt.float32, name="ps")[:, :nsz]
                        for k in range(KO):
                            nc.tensor.matmul(
                                ps,
                                xbs[blk][:, k, mi * P : (mi + 1) * P],
                                wbk[(nr, k)][:],
                                start=(k == 0),
                                stop=(k == KO - 1),
                            )
                        evict_store(ps, blk, mi, nr)

            # nr2: blocks 2-3 trickle the remaining stream first (so x2/x3's
            # last readers retire early and free the x slots for block 4),
            # then blocks 0-1 run nr2 dense.
            interleaved(2, (2, 3))
            for blk in (0, 1):
                for mi in range(NB):
                    nsz = N_SPLITS[2][1]
                    ps = psp.tile([P, 512], mybir.dt.float32, name="ps")[:, :nsz]
                    for k in range(KO):
                        nc.tensor.matmul(
                            ps,
                            xsl(blk, k, mi),
                            wbk[(2, k)][:],
                            start=(k == 0),
                            stop=(k == KO - 1),
                        )
                    evict_store(ps, blk, mi, 2)

            # ---- steady state: blocks 4..NBLK-1
            for blk in range(4, NBLK):
                xb = xp.tile([P, KO, T_BLK], mybir.dt.bfloat16, name="xb")
                xdma = nc.gpsimd.dma_start(xb[:], xq[blk])
                gate = {4: (2, 8), 5: (2, 31)}.get(blk)
                if gate is not None:
                    add_dep_helper(xdma.ins, wcast[gate].ins, sync=True,
                                   reason="pace x prefetch behind w stream")
                for mi in range(NB):
                    for nr in range(len(N_SPLITS)):
                        nsz = N_SPLITS[nr][1]
                        ps = psp.tile([P, 512], mybir.dt.float32, name="ps")[:, :nsz]
                        for k in range(KO):
                            nc.tensor.matmul(
                                ps,
                                xb[:, k, mi * P : (mi + 1) * P],
                                wbk[(nr, k)][:],
                                start=(k == 0),
                                stop=(k == KO - 1),
                            )
                        evict_store(ps, blk, mi, nr)

    nc.compile()
    return nc


def _prep_inputs(x, weight, weight_scale, bias):
    x2 = np.ascontiguousarray(x, dtype=np.float32).reshape(TOKENS, IN)
    # [blk, ki, ko, t]: xq[b, ki, ko, t] = x[b*T_BLK + t, ko*P + ki]
    xq = np.ascontiguousarray(
        x2.reshape(NBLK, T_BLK, KO, P).transpose(0, 3, 2, 1)
    )
    in_maps = []
    for c in range(N_CORES):
        lo, hi = c * OUT_SH, (c + 1) * OUT_SH
        wTc = np.ascontiguousarray(weight[lo:hi, :].astype(np.float32, copy=False).T)
        scc = np.ascontiguousarray(
            np.broadcast_to(weight_scale[lo:hi].astype(np.float32, copy=False)[None, :], (P, OUT_SH))
        )
        bic = np.ascontiguousarray(
            np.broadcast_to(bias[lo:hi].astype(np.float32, copy=False)[None, :], (P, OUT_SH))
        )
        in_maps.append({"xq": xq, "wT": wTc, "scale_rep": scc, "bias_rep": bic})
    return in_maps


def kernel(x, weight, weight_scale, bias, _trace=False):
    if "nc" not in _cache:
        _cache["nc"] = _build_program()
    nc = _cache["nc"]
    in_maps = _prep_inputs(x, weight, weight_scale, bias)
    res = bass_utils.run_bass_kernel_spmd(
        nc, in_maps, core_ids=list(range(N_CORES)), trace=_trace
    )
    _cache["last_result"] = res
    out = np.concatenate([res.results[c]["y"] for c in range(N_CORES)], axis=1)
    return out.reshape(B, S, OUT)


# revision 38
# speedup vs baseline: 1.0112x; 1.0064x over previous
"""Bass/Tile kernel for nn_BitDanceFP8ScaledLinear (column-parallel over 8 NeuronCores).

y = x @ (weight * weight_scale[:, None]).T + bias
  x: [4, 2048, 4096] f32, weight: [11008, 4096] f32, weight_scale/bias: [11008] f32

Strategy (per core c of 8):
  - weight/scale/bias sharded along out_features (1376 per core); x replicated.
  - Host-side (lossless layout prep only): x is laid out k-major per 256-token
    block as [32 blocks, 128, 32 kchunks, 256 tokens] so every x-block DMA has
    32KB-contiguous per-partition runs; weight shard transposed to wT
    [4096, 1376]; scale/bias replicated to [128, 1376].
  - Device: x blocks are DMA-loaded with an inline fp32->bf16 cast (SWDGE,
    round-to-nearest). The weight streams n-range-major in 96 [128, nsz] f32
    pieces on the Sync HWDGE FIFO, DVE-cast to persistent bf16 tiles.
    Matmuls run bf16 at full PE rate, accumulating fp32 in PSUM
    (psum[tokens=128, outF<=512] += x_tile.T @ w_piece over 32 k-chunks).
  - Startup coverage: the first 4 blocks' groups run k-interleaved 8-wide
    (4 blocks x 2 m-tiles) at each n-range, so the PE consumes each weight
    piece (~1.7us of matmul) faster than it streams (~1.1us): the PE trails
    the stream with no idle instead of stalling on the 63us weight load.
  - Epilogue per PSUM group: y_piece = psum * scale + bias on DVE (per-column
    vectors pre-replicated across partitions), stored via the ScalarE HWDGE
    queue (separate ring from the weight stream - no head-of-line blocking).
  - Host gathers: concatenate core outputs along out_features.
"""

import sys

for _p in ("/opt/trn_rl_repo", "/root/.axon_site/_ro/trn_rl_repo"):
    if _p not in sys.path:
        sys.path.insert(0, _p)

import numpy as np

import concourse.tile as tile
from concourse.tile import add_dep_helper
from concourse import bacc, bass_utils, mybir

B, S, IN, OUT = 4, 2048, 4096, 11008
N_CORES = 8
OUT_SH = OUT // N_CORES  # 1376
TOKENS = B * S  # 8192
P = 128
KO = IN // P  # 32 contraction chunks
T_BLK = 256  # tokens per x block
NBLK = TOKENS // T_BLK  # 32
NB = T_BLK // P  # m-tiles per block (2)
N_SPLITS = [(0, 512), (512, 512), (1024, 352)]  # OUT_SH split into PSUM-bank-sized pieces
EARLY = 4  # blocks covered by the startup interleave

_cache = {}


def _build_program():
    nc = bacc.Bacc("TRN2", target_bir_lowering=False, debug=False, num_devices=N_CORES)

    xq = nc.dram_tensor("xq", [NBLK, P, KO, T_BLK], mybir.dt.float32, kind="ExternalInput").ap()
    wT = nc.dram_tensor("wT", [IN, OUT_SH], mybir.dt.float32, kind="ExternalInput").ap()
    sc = nc.dram_tensor("scale_rep", [P, OUT_SH], mybir.dt.float32, kind="ExternalInput").ap()
    bi = nc.dram_tensor("bias_rep", [P, OUT_SH], mybir.dt.float32, kind="ExternalInput").ap()
    y = nc.dram_tensor("y", [TOKENS, OUT_SH], mybir.dt.float32, kind="ExternalOutput").ap()

    wT_t = wT.rearrange("(ko ki) n -> ki ko n", ki=P)  # [128, 32, 1376]

    with tile.TileContext(nc) as tc:
        with (
            tc.tile_pool(name="const", bufs=1) as const,
            tc.tile_pool(name="wstage", bufs=5) as wstage,
            tc.tile_pool(name="xp", bufs=2) as xp,
            tc.tile_pool(name="outp", bufs=5) as outp,
            tc.tile_pool(name="psum", bufs=8, space="PSUM") as psp,
        ):
            # Blocks 0-1 arrive as interleaved quarter-tiles on the otherwise
            # empty SWDGE queue (fp32->bf16 cast inline, 8KB-contiguous runs):
            # fine-grained deps let the PE start at the first w piece, and the
            # total x-early bytes (8MB) spread over the w stream keep the
            # piece-arrival rate at ~the PE consumption rate.
            QK = KO // 4  # k-chunks per quarter
            xquart = {}  # (blk, q) -> tile
            for q in range(4):
                for blk in range(2):
                    xt = xp.tile([P, QK, T_BLK], mybir.dt.bfloat16, name=f"xq_{blk}_{q}", bufs=1)
                    nc.gpsimd.dma_start(xt[:], xq[blk, :, q * QK : (q + 1) * QK, :])
                    xquart[(blk, q)] = xt

            def xslice(blk, k, mi):
                if blk < 2:
                    return xquart[(blk, k // QK)][:, k % QK, mi * P : (mi + 1) * P]
                return xbs[blk][:, k, mi * P : (mi + 1) * P]

            xbs = {}

            # Weight: n-range-major stream of 96 pieces on the Sync HWDGE
            # FIFO, staged f32 then DVE-cast into persistent bf16 tiles.
            wbk = {}
            wcast = {}

            def emit_w_range(nr):
                n0, nsz = N_SPLITS[nr]
                for k in range(KO):
                    wst = wstage.tile([P, 512], mybir.dt.float32, name="wst")
                    nc.sync.dma_start(wst[:, :nsz], wT_t[:, k, n0 : n0 + nsz])
                    wbt = const.tile([P, nsz], mybir.dt.bfloat16, name=f"wb_{nr}_{k}")
                    wcast[(nr, k)] = nc.vector.tensor_copy(wbt[:], wst[:, :nsz])
                    wbk[(nr, k)] = wbt

            emit_w_range(0)
            # scale/bias ride the ScalarE HWDGE ring, off the critical w FIFO
            sct = const.tile([P, OUT_SH], mybir.dt.float32)
            nc.scalar.dma_start(sct[:], sc[:])
            bit = const.tile([P, OUT_SH], mybir.dt.float32)
            nc.scalar.dma_start(bit[:], bi[:])
            emit_w_range(1)
            # x3 rides the Sync FIFO between nr1 and nr2 at full rate: its
            # transfer window is covered by block-2's dense work (x2-only),
            # and the nr2 pieces stream clean of SWDGE interference after it.
            xb3 = xp.tile([P, KO, T_BLK], mybir.dt.bfloat16, name="xb")
            for h in range(4):
                xst = wstage.tile([P, 8, T_BLK], mybir.dt.float32, name="x3stage", bufs=1)
                nc.sync.dma_start(xst[:], xq[3, :, h * 8 : (h + 1) * 8, :])
                nc.vector.tensor_copy(xb3[:, h * 8 : (h + 1) * 8, :], xst[:])
            xbs[3] = xb3
            emit_w_range(2)

            def evict_store(ps, blk, mi, nr):
                """y_piece = psum * scale + bias; store via ScalarE HWDGE."""
                n0, nsz = N_SPLITS[nr]
                op = outp.tile([P, 512], mybir.dt.float32, name="op")[:, :nsz]
                nc.vector.tensor_mul(op, ps, sct[:, n0 : n0 + nsz])
                nc.vector.tensor_add(op, op, bit[:, n0 : n0 + nsz])
                trow = blk * T_BLK + mi * P
                nc.scalar.dma_start(y[trow : trow + P, n0 : n0 + nsz], op)

            # x blocks 2-3: full-tile SWDGE loads gated into nr1's surplus
            # window (the 4-wide nr1 interleave over-covers the stream 2.7x,
            # so their bandwidth theft is absorbed there).
            for blk in (2,):
                xb = xp.tile([P, KO, T_BLK], mybir.dt.bfloat16, name="xb")
                xdma = nc.gpsimd.dma_start(xb[:], xq[blk])
                add_dep_helper(xdma.ins, wcast[(0, 26)].ins, sync=True,
                               reason="pace x prefetch behind w stream")
                xbs[blk] = xb

            def xsl(blk, k, mi):
                if blk < 2:
                    return xquart[(blk, k // QK)][:, k % QK, mi * P : (mi + 1) * P]
                return xbs[blk][:, k, mi * P : (mi + 1) * P]

            # ---- startup phase. nr0/nr1: blocks 0-1 k-interleaved 4-wide,
            # trailing the weight stream.
            def interleaved(nr, blocks):
                nsz = N_SPLITS[nr][1]
                groups = [(blk, mi) for blk in blocks for mi in range(NB)]
                pss = [psp.tile([P, 512], mybir.dt.float32, name="ps")[:, :nsz] for _ in groups]
                for k in range(KO):
                    for g, (blk, mi) in enumerate(groups):
                        nc.tensor.matmul(
                            pss[g],
                            xsl(blk, k, mi),
                            wbk[(nr, k)][:],
                            start=(k == 0),
                            stop=(k == KO - 1),
                        )
                for g, (blk, mi) in enumerate(groups):
                    evict_store(pss[g], blk, mi, nr)

            interleaved(0, (0, 1))
            interleaved(1, (0, 1))

            # blocks 2-3: nr0/nr1 dense while the nr2 pieces stream in — by
            # the time the nr2 interleave starts everything is resident.
            for blk in (2, 3):
                for mi in range(NB):
                    for nr in (0, 1):
                        nsz = N_SPLITS[nr][1]
                        ps = psp.tile([P, 512], mybir.dt.float32, name="ps")[:, :nsz]
                        for k in range(KO):
                            nc.tensor.matmul(
                                ps,
                                xbs[blk][:, k, mi * P : (mi + 1) * P],
                                wbk[(nr, k)][:],
                                start=(k == 0),
                                stop=(k == KO - 1),
                            )
                        evict_store(ps, blk, mi, nr)

            # nr2: blocks 2-3 trickle the remaining stream first (so x2/x3's
            # last readers retire early and free the x slots for block 4),
            # then blocks 0-1 run nr2 dense.
            interleaved(2, (2, 3))
            for blk in (0, 1):
                for mi in range(NB):
                    nsz = N_SPLITS[2][1]
                    ps = psp.tile([P, 512], mybir.dt.float32, name="ps")[:, :nsz]
                    for k in range(KO):
                        nc.tensor.matmul(
                            ps,
                            xsl(blk, k, mi),
                            wbk[(2, k)][:],
                            start=(k == 0),
                            stop=(k == KO - 1),
                        )
                    evict_store(ps, blk, mi, 2)

            # ---- steady state: blocks 4..NBLK-1
            for blk in range(4, NBLK):
                xb = xp.tile([P, KO, T_BLK], mybir.dt.bfloat16, name="xb")
                xdma = nc.gpsimd.dma_start(xb[:], xq[blk])
                gate = {4: (2, 8), 5: (2, 31)}.get(blk)
                if gate is not None:
                    add_dep_helper(xdma.ins, wcast[gate].ins, sync=True,
                                   reason="pace x prefetch behind w stream")
                for mi in range(NB):
                    for nr in range(len(N_SPLITS)):
                        nsz = N_SPLITS[nr][1]
                        ps = psp.tile([P, 512], mybir.dt.float32, name="ps")[:, :nsz]
                        for k in range(KO):
                            nc.tensor.matmul(
                                ps,
                                xb[:, k, mi * P : (mi + 1) * P],
                                wbk[(nr, k)][:],
                                start=(k == 0),
                                stop=(k == KO - 1),
                            )
                        evict_store(ps, blk, mi, nr)

    nc.compile()
    return nc


def _prep_inputs(x, weight, weight_scale, bias):
    x2 = np.ascontiguousarray(x, dtype=np.float32).reshape(TOKENS, IN)
    # [blk, ki, ko, t]: xq[b, ki, ko, t] = x[b*T_BLK + t, ko*P + ki]
    xq = np.ascontiguousarray(
        x2.reshape(NBLK, T_BLK, KO, P).transpose(0, 3, 2, 1)
    )
    in_maps = []
    for c in range(N_CORES):
        lo, hi = c * OUT_SH, (c + 1) * OUT_SH
        wTc = np.ascontiguousarray(weight[lo:hi, :].astype(np.float32, copy=False).T)
        scc = np.ascontiguousarray(
            np.broadcast_to(weight_scale[lo:hi].astype(np.float32, copy=False)[None, :], (P, OUT_SH))
        )
        bic = np.ascontiguousarray(
            np.broadcast_to(bias[lo:hi].astype(np.float32, copy=False)[None, :], (P, OUT_SH))
        )
        in_maps.append({"xq": xq, "wT": wTc, "scale_rep": scc, "bias_rep": bic})
    return in_maps


def kernel(x, weight, weight_scale, bias, _trace=False):
    if "nc" not in _cache:
        _cache["nc"] = _build_program()
    nc = _cache["nc"]
    in_maps = _prep_inputs(x, weight, weight_scale, bias)
    res = bass_utils.run_bass_kernel_spmd(
        nc, in_maps, core_ids=list(range(N_CORES)), trace=_trace
    )
    _cache["last_result"] = res
    out = np.concatenate([res.results[c]["y"] for c in range(N_CORES)], axis=1)
    return out.reshape(B, S, OUT)
